# revision 10
# baseline (speedup 1.0000x reference)
"""EntropyGate fused kernel for Trainium2 NeuronCores — transfer-optimized.

Problem (hardcoded shapes): B=4, S=4096, D=2048, window=8.
  H = entropy of softmax over sliding causal window (8) of token L2 norms of x
  gate_in = [y_ssm | y_attn | H]  (B,S,2D+1)
  h = silu(gate_in @ W1 + b1); g = sigmoid(h @ W2 + b2)
  out = g*y_ssm + (1-g)*y_attn

The axon tunnel to the cores moves ~35-40 MB/s, so wall time is transfer
bound; the design minimizes bytes on the wire:
  - y_ssm/y_attn are sent as per-token-scaled int8 (1 B/elem).
  - W1/W2 are sent as per-column-scaled int8; the column scales fold into
    the Silu/Sigmoid activation `scale` operand on device (exact).
  - The entropy feature H is computed on host (needs only token norms)
    and shipped as TOK floats per core.
  - The device returns only the gate g quantized to uint8; the host
    reconstructs out = ya + g*(ys-ya) from its full-precision inputs.
Device-side, activations arrive token-major and are PE-transposed to
feature-major for the matmuls (host transposes would dominate wall time).

Sharding: tokens (B*S = 16384) split evenly across cores; weights
replicated per core.
"""

import numpy as np
import ml_dtypes

P = 128
B, S, D = 4, 4096, 2048
T = B * S                 # total tokens
N_CORES = 1               # cores used (token shards)
TOK = T // N_CORES        # tokens per core
CH = 1024                 # token chunk processed per pass
NCH = TOK // CH
MT = D // P               # 16 output blocks of 128
KC = 2 * D // P           # 32 contraction tiles for mm1
K2 = D // P               # 16 contraction tiles for mm2
WIN = 8
W2R = 2 * D + 1           # row offset of W2 inside packed wq

# aux (f32) packing offsets, per core
A_YSC = 0
A_YAC = TOK
A_H = 2 * TOK
A_W1S = 3 * TOK
A_W2S = 3 * TOK + D
A_B1 = 3 * TOK + 2 * D
A_B2 = 3 * TOK + 3 * D
AUX = 3 * TOK + 4 * D

_BF16 = ml_dtypes.bfloat16
_NC_CACHE = {}


def _ensure_jax_cache():
    # run_bass_kernel_spmd re-jits its wrapper every call; a persistent
    # compilation cache turns that (and fresh-process recompiles) into
    # fast disk hits.
    if "jaxcache" in _NC_CACHE:
        return
    import jax

    try:
        jax.config.update("jax_compilation_cache_dir", "/tmp/eg_jax_cache")
        jax.config.update("jax_persistent_cache_min_compile_time_secs", 0.5)
    except Exception:
        pass
    _NC_CACHE["jaxcache"] = True


def _build_nc():
    import concourse.bass as bass
    import concourse.tile as tile
    import concourse.mybir as mybir
    from concourse import bacc
    from contextlib import ExitStack

    f32 = mybir.dt.float32
    bf16 = mybir.dt.bfloat16
    i8 = mybir.dt.int8
    u8 = mybir.dt.uint8
    i32 = mybir.dt.int32
    AF = mybir.ActivationFunctionType
    ALU = mybir.AluOpType

    nc = bacc.Bacc("TRN2", target_bir_lowering=False, debug=False, num_devices=1)

    yq = nc.dram_tensor("yq", [2 * TOK, D], i8, kind="ExternalInput")
    wq = nc.dram_tensor("wq", [W2R + D, D], i8, kind="ExternalInput")
    aux = nc.dram_tensor("aux", [AUX], f32, kind="ExternalInput")
    g8 = nc.dram_tensor("g8", [TOK, D], u8, kind="ExternalOutput")

    with tile.TileContext(nc) as tc:
        with ExitStack() as ctx:
            const = ctx.enter_context(tc.tile_pool(name="const", bufs=1))
            stage = ctx.enter_context(tc.tile_pool(name="stage", bufs=3))
            gatep = ctx.enter_context(tc.tile_pool(name="gatep", bufs=1))
            htp = ctx.enter_context(tc.tile_pool(name="htp", bufs=1))
            gbp = ctx.enter_context(tc.tile_pool(name="gbp", bufs=1))
            gnp = ctx.enter_context(tc.tile_pool(name="gnp", bufs=2))
            wp = ctx.enter_context(tc.tile_pool(name="wp", bufs=4))
            zp = ctx.enter_context(tc.tile_pool(name="zp", bufs=3))
            smol = ctx.enter_context(tc.tile_pool(name="smol", bufs=2))
            ps = ctx.enter_context(tc.tile_pool(name="ps", bufs=8, space="PSUM"))

            # 128x128 identity for PE transposes
            iot = const.tile([P, P], i32)
            nc.gpsimd.iota(iot[:], pattern=[[1, P]], channel_multiplier=-1)
            ident = const.tile([P, P], bf16)
            nc.vector.tensor_scalar(
                ident[:], iot[:], 0, 1.0, op0=ALU.is_equal, op1=ALU.mult
            )

            # per-partition scale/bias columns: col m holds values for
            # output block m (w1scb[p, m] = w1sc[m*128 + p], etc.)
            w1scb = const.tile([P, MT], f32)
            nc.gpsimd.dma_start(w1scb[:], bass.AP(aux, A_W1S, [[1, P], [P, MT]]))
            w2scb = const.tile([P, MT], f32)
            nc.gpsimd.dma_start(w2scb[:], bass.AP(aux, A_W2S, [[1, P], [P, MT]]))
            b1sb = const.tile([P, MT], f32)
            nc.gpsimd.dma_start(b1sb[:], bass.AP(aux, A_B1, [[1, P], [P, MT]]))
            b2sb = const.tile([P, MT], f32)
            nc.gpsimd.dma_start(b2sb[:], bass.AP(aux, A_B2, [[1, P], [P, MT]]))

            NB = CH // P   # 128-token blocks per chunk
            N2 = CH // 512  # psum n-splits per chunk

            for c in range(NCH):
                # ---- phase A: load int8 y, dequant, PE-transpose to
                # feature-major gateT tiles [128 feat, CH tok] ----
                gts = []
                for y in range(2):
                    row = [gatep.tile([P, CH], bf16, name="gt",
                                      tag=f"gt{y}_{f}") for f in range(MT)]
                    gts.append(row)
                    scoff = A_YSC if y == 0 else A_YAC
                    for r in range(NB):
                        row0 = y * TOK + c * CH + r * P
                        yt = stage.tile([P, D], i8, name="yt", tag="yt")
                        nc.sync.dma_start(yt[:], yq.ap()[row0:row0 + P, :])
                        sct = smol.tile([P, 1], f32, name="sct", tag="sct")
                        nc.gpsimd.dma_start(
                            sct[:],
                            bass.AP(aux, scoff + c * CH + r * P, [[1, P], [1, 1]]),
                        )
                        dq = stage.tile([P, D], bf16, name="dq", tag="dq")
                        nc.scalar.activation(dq[:], yt[:], AF.Copy,
                                             scale=sct[:, 0:1])
                        for f in range(MT):
                            pt = ps.tile([P, P], bf16, name="ptr", tag="pt")
                            nc.tensor.transpose(
                                pt[:], dq[:, f * P:(f + 1) * P], ident[:]
                            )
                            nc.vector.tensor_copy(
                                gts[y][f][:, r * P:(r + 1) * P], pt[:]
                            )
                gflat = gts[0] + gts[1]

                # H feature row for this chunk (f32 -> bf16 on device)
                hrf = smol.tile([1, CH], f32, name="hrf", tag="hrf")
                nc.gpsimd.dma_start(
                    hrf[:], bass.AP(aux, A_H + c * CH, [[CH, 1], [1, CH]])
                )
                hrow = smol.tile([1, CH], bf16, name="hrow", tag="hrow")
                nc.scalar.activation(hrow[:], hrf[:], AF.Copy)

                # ---- mm1: hT[m, tok] = silu(s1[m]*(W1raw.T @ gateT) + b1) ----
                hts = [htp.tile([P, CH], bf16, name="ht", tag=f"ht{m}")
                       for m in range(MT)]
                for mg in range(4):
                    csl = slice(mg * 512, (mg + 1) * 512)
                    pts = [[ps.tile([P, 512], f32, name="pt1", tag="pt")
                            for _ in range(N2)] for _ in range(4)]
                    wH8 = wp.tile([1, 512], i8, name="wH8", tag="wH8")
                    nc.sync.dma_start(wH8[:], wq.ap()[2 * D:2 * D + 1, csl])
                    wH = wp.tile([1, 512], bf16, name="wH", tag="wH")
                    nc.scalar.activation(wH[:], wH8[:], AF.Copy)
                    for k in range(KC):
                        w8 = wp.tile([P, 512], i8, name="w8", tag="w8")
                        nc.sync.dma_start(w8[:], wq.ap()[k * P:(k + 1) * P, csl])
                        wb = wp.tile([P, 512], bf16, name="wb", tag="wb")
                        nc.scalar.activation(wb[:], w8[:], AF.Copy)
                        for mi in range(4):
                            for n in range(N2):
                                nc.tensor.matmul(
                                    pts[mi][n][:],
                                    wb[:, mi * P:(mi + 1) * P],
                                    gflat[k][:, n * 512:(n + 1) * 512],
                                    start=(k == 0), stop=False,
                                )
                    for mi in range(4):
                        m = mg * 4 + mi
                        for n in range(N2):
                            nc.tensor.matmul(
                                pts[mi][n][:],
                                wH[:, mi * P:(mi + 1) * P],
                                hrow[:, n * 512:(n + 1) * 512],
                                start=False, stop=True,
                            )
                            # silu(z) = z * sigmoid(z), z = s1[m]*psum + b1[m]
                            zt = zp.tile([P, 512], f32, name="zt", tag="zt")
                            nc.scalar.activation(
                                zt[:], pts[mi][n][:], AF.Identity,
                                bias=b1sb[:, m:m + 1], scale=w1scb[:, m:m + 1],
                            )
                            sg = zp.tile([P, 512], f32, name="sg", tag="sg")
                            nc.scalar.activation(sg[:], zt[:], AF.Sigmoid)
                            nc.vector.tensor_mul(
                                hts[m][:, n * 512:(n + 1) * 512], zt[:], sg[:]
                            )

                # ---- mm2: gT[e, tok] = sigmoid(s2[e]*(W2raw.T @ hT) + b2) ----
                gbs = [gbp.tile([P, CH], bf16, name="gb", tag=f"gb{e}")
                       for e in range(MT)]
                for eg in range(4):
                    esl = slice(eg * 512, (eg + 1) * 512)
                    pts2 = [[ps.tile([P, 512], f32, name="pt2", tag="pt")
                             for _ in range(N2)] for _ in range(4)]
                    for k2 in range(K2):
                        w28 = wp.tile([P, 512], i8, name="w28", tag="w8")
                        nc.sync.dma_start(
                            w28[:], wq.ap()[W2R + k2 * P:W2R + (k2 + 1) * P, esl]
                        )
                        w2b = wp.tile([P, 512], bf16, name="w2b", tag="wb")
                        nc.scalar.activation(w2b[:], w28[:], AF.Copy)
                        for ei in range(4):
                            for n in range(N2):
                                nc.tensor.matmul(
                                    pts2[ei][n][:],
                                    w2b[:, ei * P:(ei + 1) * P],
                                    hts[k2][:, n * 512:(n + 1) * 512],
                                    start=(k2 == 0), stop=(k2 == K2 - 1),
                                )
                    for ei in range(4):
                        e = eg * 4 + ei
                        for n in range(N2):
                            nc.scalar.activation(
                                gbs[e][:, n * 512:(n + 1) * 512],
                                pts2[ei][n][:], AF.Sigmoid,
                                bias=b2sb[:, e:e + 1], scale=w2scb[:, e:e + 1],
                            )

                # ---- phase D: transpose g back to token-major, quantize
                # to u8 (conversion truncates, +0.5 rounds), store ----
                for r in range(NB):
                    gn = gnp.tile([P, D], u8, name="gn", tag="gn")
                    for e in range(MT):
                        ptg = ps.tile([P, P], bf16, name="ptg", tag="pt")
                        nc.tensor.transpose(
                            ptg[:], gbs[e][:, r * P:(r + 1) * P], ident[:]
                        )
                        nc.vector.tensor_scalar(
                            gn[:, e * P:(e + 1) * P], ptg[:], 255.0, 0.5,
                            op0=ALU.mult, op1=ALU.add,
                        )
                    nc.sync.dma_start(
                        g8.ap()[c * CH + r * P:c * CH + (r + 1) * P, :], gn[:]
                    )
    nc.finalize()
    return nc


def _get_nc():
    if "nc" not in _NC_CACHE:
        _NC_CACHE["nc"] = _build_nc()
    return _NC_CACHE["nc"]


def _entropy_host(x2d):
    # token L2 norms -> sliding causal window softmax entropy, (T,) f32
    m = np.sqrt(np.einsum("sd,sd->s", x2d, x2d)).reshape(B, S)
    off = np.arange(WIN) - (WIN - 1)
    idx = np.arange(S)[:, None] + off[None, :]
    valid = idx >= 0
    idxc = np.clip(idx, 0, S - 1)
    wins = m[:, idxc]
    wins = np.where(valid[None], wins, -np.inf)
    wmax = wins.max(-1, keepdims=True)
    e = np.exp(wins - wmax)
    p = e / e.sum(-1, keepdims=True)
    H = -(p * np.log2(p + 1e-9)).sum(-1)
    return np.ascontiguousarray(H.reshape(-1).astype(np.float32))


def _quant_rows_into(a, out_i8):
    # per-row symmetric int8: returns scales (rows,) f32
    hi = a.max(axis=1)
    lo = a.min(axis=1)
    s = np.maximum(hi, -lo)
    s /= 127.0
    np.maximum(s, 1e-30, out=s)
    inv = 1.0 / s
    tmp = a * inv[:, None]
    np.rint(tmp, out=tmp)
    out_i8[:] = tmp
    return s.astype(np.float32)


def _quant_cols(w):
    # per-column symmetric int8: returns (q, scales (cols,) f32)
    aw = np.abs(w).max(axis=0)
    s = np.maximum(aw / 127.0, 1e-30).astype(np.float32)
    tmp = w * (1.0 / s)[None, :]
    np.rint(tmp, out=tmp)
    return tmp.astype(np.int8), s


def _make_in_maps(y_ssm, y_attn, x, W1, b1, W2, b2):
    ys = np.asarray(y_ssm, np.float32).reshape(T, D)
    ya = np.asarray(y_attn, np.float32).reshape(T, D)
    xs = np.asarray(x, np.float32).reshape(T, D)
    W1f = np.asarray(W1, np.float32)
    W2f = np.asarray(W2, np.float32)
    b1f = np.asarray(b1, np.float32)
    b2f = np.asarray(b2, np.float32)

    Hent = _entropy_host(xs)

    w1q, w1s = _quant_cols(W1f)
    w2q, w2s = _quant_cols(W2f)
    wq = np.concatenate([w1q, w2q], axis=0)  # (2D+1+D, D): W2 rows start at W2R
    assert wq.shape[0] == W2R + D

    in_maps = []
    for c in range(N_CORES):
        t0 = c * TOK
        yq_c = np.empty((2 * TOK, D), np.int8)
        ysc = _quant_rows_into(ys[t0:t0 + TOK], yq_c[:TOK])
        yac = _quant_rows_into(ya[t0:t0 + TOK], yq_c[TOK:])
        aux = np.empty(AUX, np.float32)
        aux[A_YSC:A_YSC + TOK] = ysc
        aux[A_YAC:A_YAC + TOK] = yac
        aux[A_H:A_H + TOK] = Hent[t0:t0 + TOK]
        aux[A_W1S:A_W1S + D] = w1s
        aux[A_W2S:A_W2S + D] = w2s
        aux[A_B1:A_B1 + D] = b1f
        aux[A_B2:A_B2 + D] = b2f
        in_maps.append({"yq": yq_c, "wq": wq, "aux": aux})
    return in_maps, ys, ya


def _run(in_maps, trace=False):
    from concourse.bass_utils import run_bass_kernel_spmd
    _ensure_jax_cache()
    nc = _get_nc()
    return run_bass_kernel_spmd(
        nc, in_maps, core_ids=list(range(N_CORES)), trace=trace
    )


def _recon(g8, ys, ya):
    # out = ya + (g8/255)*(ys-ya), fused on jax-cpu (multithreaded, one pass)
    import jax

    if "recon" not in _NC_CACHE:
        import jax.numpy as jnp

        @jax.jit
        def f(g8, ys, ya):
            g = g8.astype(jnp.float32) * np.float32(1.0 / 255.0)
            return ya + g * (ys - ya)

        _NC_CACHE["recon"] = f
    cpu = jax.devices("cpu")[0]
    with jax.default_device(cpu):
        out = _NC_CACHE["recon"](g8, ys, ya)
    return np.asarray(out)


def kernel(y_ssm, y_attn, x, W1, b1, W2, b2):
    in_maps, ys, ya = _make_in_maps(y_ssm, y_attn, x, W1, b1, W2, b2)
    res = _run(in_maps, trace=False)
    g = np.concatenate([r["g8"] for r in res.results], axis=0)  # (T, D) u8
    return _recon(g, ys, ya).reshape(B, S, D)


# revision 11
# speedup vs baseline: 1.0457x; 1.0457x over previous
"""EntropyGate fused kernel for Trainium2 NeuronCores — transfer-optimized.

Problem (hardcoded shapes): B=4, S=4096, D=2048, window=8.
  H = entropy of softmax over sliding causal window (8) of token L2 norms of x
  gate_in = [y_ssm | y_attn | H]  (B,S,2D+1)
  h = silu(gate_in @ W1 + b1); g = sigmoid(h @ W2 + b2)
  out = g*y_ssm + (1-g)*y_attn

The axon tunnel to the cores moves ~35-40 MB/s, so wall time is transfer
bound; the design minimizes bytes on the wire:
  - y_ssm/y_attn are sent as per-token-scaled int8 (1 B/elem).
  - W1/W2 are sent as per-column-scaled int8; the column scales fold into
    the Silu/Sigmoid activation `scale` operand on device (exact).
  - The entropy feature H is computed on host (needs only token norms)
    and shipped as TOK floats per core.
  - The device returns only the gate g quantized to uint8; the host
    reconstructs out = ya + g*(ys-ya) from its full-precision inputs.
Device-side, activations arrive token-major and are PE-transposed to
feature-major for the matmuls (host transposes would dominate wall time).

Sharding: tokens (B*S = 16384) split evenly across cores; weights
replicated per core.
"""

import numpy as np
import ml_dtypes

P = 128
B, S, D = 4, 4096, 2048
T = B * S                 # total tokens
N_CORES = 2               # cores used (token shards)
TOK = T // N_CORES        # tokens per core
CH = 1024                 # token chunk processed per pass
NCH = TOK // CH
MT = D // P               # 16 output blocks of 128
KC = 2 * D // P           # 32 contraction tiles for mm1
K2 = D // P               # 16 contraction tiles for mm2
WIN = 8
W2R = 2 * D + 1           # row offset of W2 inside packed wq

# aux (f32) packing offsets, per core
A_YSC = 0
A_YAC = TOK
A_H = 2 * TOK
A_W1S = 3 * TOK
A_W2S = 3 * TOK + D
A_B1 = 3 * TOK + 2 * D
A_B2 = 3 * TOK + 3 * D
AUX = 3 * TOK + 4 * D

_BF16 = ml_dtypes.bfloat16
_NC_CACHE = {}


def _ensure_jax_cache():
    # run_bass_kernel_spmd re-jits its wrapper every call; a persistent
    # compilation cache turns that (and fresh-process recompiles) into
    # fast disk hits.
    if "jaxcache" in _NC_CACHE:
        return
    import jax

    try:
        jax.config.update("jax_compilation_cache_dir", "/tmp/eg_jax_cache")
        jax.config.update("jax_persistent_cache_min_compile_time_secs", 0.5)
    except Exception:
        pass
    _NC_CACHE["jaxcache"] = True


def _build_nc():
    import concourse.bass as bass
    import concourse.tile as tile
    import concourse.mybir as mybir
    from concourse import bacc
    from contextlib import ExitStack

    f32 = mybir.dt.float32
    bf16 = mybir.dt.bfloat16
    i8 = mybir.dt.int8
    u8 = mybir.dt.uint8
    i32 = mybir.dt.int32
    AF = mybir.ActivationFunctionType
    ALU = mybir.AluOpType

    nc = bacc.Bacc("TRN2", target_bir_lowering=False, debug=False, num_devices=1)

    yq = nc.dram_tensor("yq", [2 * TOK, D], i8, kind="ExternalInput")
    wq = nc.dram_tensor("wq", [W2R + D, D], i8, kind="ExternalInput")
    aux = nc.dram_tensor("aux", [AUX], f32, kind="ExternalInput")
    g8 = nc.dram_tensor("g8", [TOK, D], u8, kind="ExternalOutput")

    with tile.TileContext(nc) as tc:
        with ExitStack() as ctx:
            const = ctx.enter_context(tc.tile_pool(name="const", bufs=1))
            stage = ctx.enter_context(tc.tile_pool(name="stage", bufs=3))
            gatep = ctx.enter_context(tc.tile_pool(name="gatep", bufs=1))
            htp = ctx.enter_context(tc.tile_pool(name="htp", bufs=1))
            gbp = ctx.enter_context(tc.tile_pool(name="gbp", bufs=1))
            gnp = ctx.enter_context(tc.tile_pool(name="gnp", bufs=2))
            wp = ctx.enter_context(tc.tile_pool(name="wp", bufs=4))
            zp = ctx.enter_context(tc.tile_pool(name="zp", bufs=3))
            smol = ctx.enter_context(tc.tile_pool(name="smol", bufs=2))
            ps = ctx.enter_context(tc.tile_pool(name="ps", bufs=8, space="PSUM"))

            # 128x128 identity for PE transposes
            iot = const.tile([P, P], i32)
            nc.gpsimd.iota(iot[:], pattern=[[1, P]], channel_multiplier=-1)
            ident = const.tile([P, P], bf16)
            nc.vector.tensor_scalar(
                ident[:], iot[:], 0, 1.0, op0=ALU.is_equal, op1=ALU.mult
            )

            # per-partition scale/bias columns: col m holds values for
            # output block m (w1scb[p, m] = w1sc[m*128 + p], etc.)
            w1scb = const.tile([P, MT], f32)
            nc.gpsimd.dma_start(w1scb[:], bass.AP(aux, A_W1S, [[1, P], [P, MT]]))
            w2scb = const.tile([P, MT], f32)
            nc.gpsimd.dma_start(w2scb[:], bass.AP(aux, A_W2S, [[1, P], [P, MT]]))
            b1sb = const.tile([P, MT], f32)
            nc.gpsimd.dma_start(b1sb[:], bass.AP(aux, A_B1, [[1, P], [P, MT]]))
            b2sb = const.tile([P, MT], f32)
            nc.gpsimd.dma_start(b2sb[:], bass.AP(aux, A_B2, [[1, P], [P, MT]]))

            NB = CH // P   # 128-token blocks per chunk
            N2 = CH // 512  # psum n-splits per chunk

            for c in range(NCH):
                # ---- phase A: load int8 y, dequant, PE-transpose to
                # feature-major gateT tiles [128 feat, CH tok] ----
                gts = []
                for y in range(2):
                    row = [gatep.tile([P, CH], bf16, name="gt",
                                      tag=f"gt{y}_{f}") for f in range(MT)]
                    gts.append(row)
                    scoff = A_YSC if y == 0 else A_YAC
                    for r in range(NB):
                        row0 = y * TOK + c * CH + r * P
                        yt = stage.tile([P, D], i8, name="yt", tag="yt")
                        nc.sync.dma_start(yt[:], yq.ap()[row0:row0 + P, :])
                        sct = smol.tile([P, 1], f32, name="sct", tag="sct")
                        nc.gpsimd.dma_start(
                            sct[:],
                            bass.AP(aux, scoff + c * CH + r * P, [[1, P], [1, 1]]),
                        )
                        dq = stage.tile([P, D], bf16, name="dq", tag="dq")
                        nc.scalar.activation(dq[:], yt[:], AF.Copy,
                                             scale=sct[:, 0:1])
                        for f in range(MT):
                            pt = ps.tile([P, P], bf16, name="ptr", tag="pt")
                            nc.tensor.transpose(
                                pt[:], dq[:, f * P:(f + 1) * P], ident[:]
                            )
                            nc.vector.tensor_copy(
                                gts[y][f][:, r * P:(r + 1) * P], pt[:]
                            )
                gflat = gts[0] + gts[1]

                # H feature row for this chunk (f32 -> bf16 on device)
                hrf = smol.tile([1, CH], f32, name="hrf", tag="hrf")
                nc.gpsimd.dma_start(
                    hrf[:], bass.AP(aux, A_H + c * CH, [[CH, 1], [1, CH]])
                )
                hrow = smol.tile([1, CH], bf16, name="hrow", tag="hrow")
                nc.scalar.activation(hrow[:], hrf[:], AF.Copy)

                # ---- mm1: hT[m, tok] = silu(s1[m]*(W1raw.T @ gateT) + b1) ----
                hts = [htp.tile([P, CH], bf16, name="ht", tag=f"ht{m}")
                       for m in range(MT)]
                for mg in range(4):
                    csl = slice(mg * 512, (mg + 1) * 512)
                    pts = [[ps.tile([P, 512], f32, name="pt1", tag="pt")
                            for _ in range(N2)] for _ in range(4)]
                    wH8 = wp.tile([1, 512], i8, name="wH8", tag="wH8")
                    nc.sync.dma_start(wH8[:], wq.ap()[2 * D:2 * D + 1, csl])
                    wH = wp.tile([1, 512], bf16, name="wH", tag="wH")
                    nc.scalar.activation(wH[:], wH8[:], AF.Copy)
                    for k in range(KC):
                        w8 = wp.tile([P, 512], i8, name="w8", tag="w8")
                        nc.sync.dma_start(w8[:], wq.ap()[k * P:(k + 1) * P, csl])
                        wb = wp.tile([P, 512], bf16, name="wb", tag="wb")
                        nc.scalar.activation(wb[:], w8[:], AF.Copy)
                        for mi in range(4):
                            for n in range(N2):
                                nc.tensor.matmul(
                                    pts[mi][n][:],
                                    wb[:, mi * P:(mi + 1) * P],
                                    gflat[k][:, n * 512:(n + 1) * 512],
                                    start=(k == 0), stop=False,
                                )
                    for mi in range(4):
                        m = mg * 4 + mi
                        for n in range(N2):
                            nc.tensor.matmul(
                                pts[mi][n][:],
                                wH[:, mi * P:(mi + 1) * P],
                                hrow[:, n * 512:(n + 1) * 512],
                                start=False, stop=True,
                            )
                            # silu(z) = z * sigmoid(z), z = s1[m]*psum + b1[m]
                            zt = zp.tile([P, 512], f32, name="zt", tag="zt")
                            nc.scalar.activation(
                                zt[:], pts[mi][n][:], AF.Identity,
                                bias=b1sb[:, m:m + 1], scale=w1scb[:, m:m + 1],
                            )
                            sg = zp.tile([P, 512], f32, name="sg", tag="sg")
                            nc.scalar.activation(sg[:], zt[:], AF.Sigmoid)
                            nc.vector.tensor_mul(
                                hts[m][:, n * 512:(n + 1) * 512], zt[:], sg[:]
                            )

                # ---- mm2: gT[e, tok] = sigmoid(s2[e]*(W2raw.T @ hT) + b2) ----
                gbs = [gbp.tile([P, CH], bf16, name="gb", tag=f"gb{e}")
                       for e in range(MT)]
                for eg in range(4):
                    esl = slice(eg * 512, (eg + 1) * 512)
                    pts2 = [[ps.tile([P, 512], f32, name="pt2", tag="pt")
                             for _ in range(N2)] for _ in range(4)]
                    for k2 in range(K2):
                        w28 = wp.tile([P, 512], i8, name="w28", tag="w8")
                        nc.sync.dma_start(
                            w28[:], wq.ap()[W2R + k2 * P:W2R + (k2 + 1) * P, esl]
                        )
                        w2b = wp.tile([P, 512], bf16, name="w2b", tag="wb")
                        nc.scalar.activation(w2b[:], w28[:], AF.Copy)
                        for ei in range(4):
                            for n in range(N2):
                                nc.tensor.matmul(
                                    pts2[ei][n][:],
                                    w2b[:, ei * P:(ei + 1) * P],
                                    hts[k2][:, n * 512:(n + 1) * 512],
                                    start=(k2 == 0), stop=(k2 == K2 - 1),
                                )
                    for ei in range(4):
                        e = eg * 4 + ei
                        for n in range(N2):
                            nc.scalar.activation(
                                gbs[e][:, n * 512:(n + 1) * 512],
                                pts2[ei][n][:], AF.Sigmoid,
                                bias=b2sb[:, e:e + 1], scale=w2scb[:, e:e + 1],
                            )

                # ---- phase D: transpose g back to token-major, quantize
                # to u8 (conversion truncates, +0.5 rounds), store ----
                for r in range(NB):
                    gn = gnp.tile([P, D], u8, name="gn", tag="gn")
                    for e in range(MT):
                        ptg = ps.tile([P, P], bf16, name="ptg", tag="pt")
                        nc.tensor.transpose(
                            ptg[:], gbs[e][:, r * P:(r + 1) * P], ident[:]
                        )
                        nc.vector.tensor_scalar(
                            gn[:, e * P:(e + 1) * P], ptg[:], 255.0, 0.5,
                            op0=ALU.mult, op1=ALU.add,
                        )
                    nc.sync.dma_start(
                        g8.ap()[c * CH + r * P:c * CH + (r + 1) * P, :], gn[:]
                    )
    nc.finalize()
    return nc


def _get_nc():
    if "nc" not in _NC_CACHE:
        _NC_CACHE["nc"] = _build_nc()
    return _NC_CACHE["nc"]


def _entropy_host(x2d):
    # token L2 norms -> sliding causal window softmax entropy, (T,) f32
    m = np.sqrt(np.einsum("sd,sd->s", x2d, x2d)).reshape(B, S)
    off = np.arange(WIN) - (WIN - 1)
    idx = np.arange(S)[:, None] + off[None, :]
    valid = idx >= 0
    idxc = np.clip(idx, 0, S - 1)
    wins = m[:, idxc]
    wins = np.where(valid[None], wins, -np.inf)
    wmax = wins.max(-1, keepdims=True)
    e = np.exp(wins - wmax)
    p = e / e.sum(-1, keepdims=True)
    H = -(p * np.log2(p + 1e-9)).sum(-1)
    return np.ascontiguousarray(H.reshape(-1).astype(np.float32))


def _quant_rows_into(a, out_i8):
    # per-row symmetric int8: returns scales (rows,) f32
    hi = a.max(axis=1)
    lo = a.min(axis=1)
    s = np.maximum(hi, -lo)
    s /= 127.0
    np.maximum(s, 1e-30, out=s)
    inv = 1.0 / s
    tmp = a * inv[:, None]
    np.rint(tmp, out=tmp)
    out_i8[:] = tmp
    return s.astype(np.float32)


def _quant_cols(w):
    # per-column symmetric int8: returns (q, scales (cols,) f32)
    aw = np.abs(w).max(axis=0)
    s = np.maximum(aw / 127.0, 1e-30).astype(np.float32)
    tmp = w * (1.0 / s)[None, :]
    np.rint(tmp, out=tmp)
    return tmp.astype(np.int8), s


def _make_in_maps(y_ssm, y_attn, x, W1, b1, W2, b2):
    ys = np.asarray(y_ssm, np.float32).reshape(T, D)
    ya = np.asarray(y_attn, np.float32).reshape(T, D)
    xs = np.asarray(x, np.float32).reshape(T, D)
    W1f = np.asarray(W1, np.float32)
    W2f = np.asarray(W2, np.float32)
    b1f = np.asarray(b1, np.float32)
    b2f = np.asarray(b2, np.float32)

    Hent = _entropy_host(xs)

    w1q, w1s = _quant_cols(W1f)
    w2q, w2s = _quant_cols(W2f)
    wq = np.concatenate([w1q, w2q], axis=0)  # (2D+1+D, D): W2 rows start at W2R
    assert wq.shape[0] == W2R + D

    in_maps = []
    for c in range(N_CORES):
        t0 = c * TOK
        yq_c = np.empty((2 * TOK, D), np.int8)
        ysc = _quant_rows_into(ys[t0:t0 + TOK], yq_c[:TOK])
        yac = _quant_rows_into(ya[t0:t0 + TOK], yq_c[TOK:])
        aux = np.empty(AUX, np.float32)
        aux[A_YSC:A_YSC + TOK] = ysc
        aux[A_YAC:A_YAC + TOK] = yac
        aux[A_H:A_H + TOK] = Hent[t0:t0 + TOK]
        aux[A_W1S:A_W1S + D] = w1s
        aux[A_W2S:A_W2S + D] = w2s
        aux[A_B1:A_B1 + D] = b1f
        aux[A_B2:A_B2 + D] = b2f
        in_maps.append({"yq": yq_c, "wq": wq, "aux": aux})
    return in_maps, ys, ya


def _run(in_maps, trace=False):
    from concourse.bass_utils import run_bass_kernel_spmd
    _ensure_jax_cache()
    nc = _get_nc()
    return run_bass_kernel_spmd(
        nc, in_maps, core_ids=list(range(N_CORES)), trace=trace
    )


def _recon(g8, ys, ya):
    # out = ya + (g8/255)*(ys-ya), fused on jax-cpu (multithreaded, one pass)
    import jax

    if "recon" not in _NC_CACHE:
        import jax.numpy as jnp

        @jax.jit
        def f(g8, ys, ya):
            g = g8.astype(jnp.float32) * np.float32(1.0 / 255.0)
            return ya + g * (ys - ya)

        _NC_CACHE["recon"] = f
    cpu = jax.devices("cpu")[0]
    with jax.default_device(cpu):
        out = _NC_CACHE["recon"](g8, ys, ya)
    return np.asarray(out)


def kernel(y_ssm, y_attn, x, W1, b1, W2, b2):
    in_maps, ys, ya = _make_in_maps(y_ssm, y_attn, x, W1, b1, W2, b2)
    res = _run(in_maps, trace=False)
    g = np.concatenate([r["g8"] for r in res.results], axis=0)  # (T, D) u8
    return _recon(g, ys, ya).reshape(B, S, D)
